# revision 5
# baseline (speedup 1.0000x reference)
"""TRN2 Bass kernel for nn_DeepGen (GNN message passing), 8 NeuronCores.

Strategy (hardcoded for N=50000, E=1600000, G=512, H=128, DEPTH=3):
 - Nodes sharded across 8 cores at graph-aligned boundaries (64 graphs/core),
   padded to NMAX=6656 rows/core; edges assigned to the owner of dst.
 - GAT softmax trick: within a dst segment the dst-side logit term is constant
   and cancels, so alpha = es[src]/sum(es[src]) with es = exp(nf @ a_src).
   Each layer's message pass is: gather rows [es*nf, es] (bf16) by src via
   dma_gather, one-hot(dst) matmul accumulation into PSUM per 128-dst chunk,
   then divide.  Dense matmuls run in transposed layout (features on
   partitions).
 - Layer-0 node table computed replicated on all cores (no collective);
   layers 1,2 tables are AllGathered compact [NMAX,129] then expanded into the
   512B-stride gather table.
 - Attention pooling uses host-precomputed per-tile graph one-hot matrices
   (graphs are contiguous since batch is sorted); heads use BN via allreduced
   moment sums.

The module is self-contained: it builds the Bass program at call time from the
actual edge structure and runs it on cores 0..7 via run_bass_kernel_spmd.
"""
import os
import sys
import types

import numpy as np
import ml_dtypes

import concourse.bass as bass
import concourse.mybir as mybir
import concourse.tile as tile
from concourse import bacc, bass_utils

P = 128
NCORES = 8
N, E, G = 50000, 1600000, 512
H, NF_IN, NF_OUT, GF_OUT, DEPTH = 128, 64, 64, 128, 3
GPC = G // NCORES              # graphs per core
NMAX = 6656                    # padded nodes per core (13*512)
NBLK = NMAX // 512             # 13
NTILES = NMAX // P             # 52
NCHUNK = NTILES                # dst chunks per core
NTOT = NCORES * NMAX           # padded table rows
SPLIT = 32768                  # int16 index split
WT = 256                       # table row width (bf16) -> 512B rows
WC = 129                       # useful cols: [es*nf(128), es]
SUBMAX = 16                    # max subtiles (of 128 edges) per dma_gather
NQ = 4                         # swdge queues
EPS = 1e-5

f32 = mybir.dt.float32
f32r = mybir.dt.float32r
bf16 = mybir.dt.bfloat16
i16 = mybir.dt.int16
AF = mybir.ActivationFunctionType
AL = mybir.AluOpType

LAST_EXEC_NS = None


def _install_hw_shims():
    """Enable NTFF tracing + disable artifact upload under axon (best-effort)."""
    try:
        import antenv.axon_hooks  # noqa: F401
    except ImportError:
        try:
            from trn_agent_boot.trn_boot import _ntff_profile_via_ctypes
            hook = _ntff_profile_via_ctypes("/opt/axon/libaxon_pjrt.so")
            mod = types.ModuleType("antenv.axon_hooks")
            mod._hook = hook
            mod.get_axon_ntff_profile_hook = lambda: mod._hook
            mod.set_axon_ntff_profile_hook = lambda h: setattr(mod, "_hook", h)
            import antenv
            antenv.axon_hooks = mod
            sys.modules["antenv.axon_hooks"] = mod
        except Exception:
            pass
    try:
        bass_utils.upload_artifacts = lambda tmpdir: "local://" + tmpdir
    except Exception:
        pass


def _schedule(edge_index, batch):
    """Host-side edge partitioning and uniform SPMD schedule."""
    src = np.asarray(edge_index[0], np.int64)
    dst = np.asarray(edge_index[1], np.int64)
    batch = np.asarray(batch, np.int64)

    gb = np.searchsorted(batch, np.arange(0, G + 1, GPC)).astype(np.int64)  # [9]
    n_c = np.diff(gb)
    assert n_c.max() <= NMAX, f"shard too big: {n_c.max()}"

    owner_src = np.searchsorted(gb, src, side="right") - 1
    owner_dst = np.searchsorted(gb, dst, side="right") - 1
    spid = (src - gb[owner_src] + owner_src * NMAX).astype(np.int64)
    dloc = (dst - gb[owner_dst]).astype(np.int64)
    chunk = dloc >> 7
    slot = dloc & 127
    hi = (spid >= SPLIT).astype(np.int64)

    # group key: (core, chunk, half); stable order within groups
    key = (owner_dst * NCHUNK + chunk) * 2 + hi
    order = np.argsort(key, kind="stable")
    nkey = NCORES * NCHUNK * 2
    counts = np.bincount(key, minlength=nkey)
    gstartk = np.zeros(nkey + 1, np.int64)
    np.cumsum(counts, out=gstartk[1:])
    within = np.empty(E, np.int64)
    within[order] = np.arange(E) - gstartk[key[order]]

    cnt3 = counts.reshape(NCORES, NCHUNK, 2)
    # uniform per-chunk subtile counts (max over cores)
    LK = np.maximum(1, -(-cnt3[:, :, 0].max(axis=0) // P))   # [NCHUNK]
    HK = np.maximum(1, -(-cnt3[:, :, 1].max(axis=0) // P))
    sub_base = np.zeros((NCHUNK, 2), np.int64)
    acc = 0
    chunk_meta = []  # per chunk: (first_sub, nsub_total, instrs=[(half, nsub)])
    for k in range(NCHUNK):
        sub_base[k, 0] = acc
        sub_base[k, 1] = acc + LK[k]
        instrs = []
        for half, cnt in ((0, int(LK[k])), (1, int(HK[k]))):
            left = cnt
            while left > 0:
                take = min(left, SUBMAX)
                instrs.append((half, take))
                left -= take
        chunk_meta.append((acc, int(LK[k] + HK[k]), instrs))
        acc += int(LK[k] + HK[k])
    S = acc  # total subtiles per core per layer

    # per-core streams
    idx_streams, dstl_streams = [], []
    stream_pos = sub_base[chunk, hi] * P + within  # [E] position within core stream
    for c in range(NCORES):
        m = owner_dst == c
        idxs = np.zeros(S * P, np.int64)   # pad gathers row 0 of its half
        dstl = np.full(S * P, -1.0, np.float32)
        sp = stream_pos[m]
        iv = spid[m] - hi[m] * SPLIT
        idxs[sp] = iv
        dstl[sp] = slot[m].astype(np.float32)
        assert idxs.max() < SPLIT
        arr = idxs.reshape(S, P).astype(np.int16)
        B = arr.reshape(S, 8, 16).transpose(2, 0, 1).reshape(16, S * 8)
        idx16 = np.tile(B, (8, 1))
        dstlT = dstl.reshape(S, P).T.astype(ml_dtypes.bfloat16)
        idx_streams.append(np.ascontiguousarray(idx16))
        dstl_streams.append(np.ascontiguousarray(dstlT))

    # per-core masks / pooling one-hots
    invmsk, Ptiles, PTtiles = [], [], []
    for c in range(NCORES):
        nc_ = int(n_c[c])
        im = np.zeros((P, NCHUNK), np.float32)
        loc = np.arange(P)[:, None] + np.arange(NCHUNK)[None, :] * P
        im[loc >= nc_] = 1.0
        invmsk.append(im)
        gw = np.full(NMAX, -1, np.int64)
        gw[:nc_] = batch[gb[c]:gb[c + 1]] - c * GPC
        Pt = np.zeros((P, NTILES * P), np.float32)
        for t in range(NTILES):
            g_t = gw[t * P:(t + 1) * P]
            valid = g_t >= 0
            Pt[np.arange(P)[valid], t * P + g_t[valid]] = 1.0
        PTt = np.zeros_like(Pt)
        for t in range(NTILES):
            PTt[:, t * P:(t + 1) * P] = Pt[:, t * P:(t + 1) * P].T
        Ptiles.append(Pt.astype(ml_dtypes.bfloat16))
        PTtiles.append(PTt.astype(ml_dtypes.bfloat16))

    return dict(gb=gb, n_c=n_c, S=S, chunk_meta=chunk_meta,
                idx_streams=idx_streams, dstl_streams=dstl_streams,
                invmsk=invmsk, Ptiles=Ptiles, PTtiles=PTtiles)


def _build(sched):
    S = sched["S"]
    chunk_meta = sched["chunk_meta"]

    nc = bacc.Bacc("TRN2", target_bir_lowering=False, debug=False,
                   enable_asserts=False, num_devices=NCORES,
                   num_swdge_queues=NQ)

    def din(name, shape, dt):
        return nc.dram_tensor(name, shape, dt, kind="ExternalInput").ap()

    xT_aug = din("xT_aug", [NF_IN + 1, NTOT], f32)
    xT_own = din("xT_own", [NF_IN + 1, NMAX], f32)
    idx_d = din("idx16", [P, S * 8], i16)
    dstl_d = din("dstl", [P, S], bf16)
    invmsk_d = din("invmsk", [P, NCHUNK], f32)
    P_d = din("Pt", [P, NTILES * P], bf16)
    PT_d = din("PTt", [P, NTILES * P], bf16)
    iota_d = din("iota", [P, P], bf16)
    ident_d = din("ident", [P, P], f32)
    ones_d = din("ones", [P, 1], f32)
    WiT_d = din("WiT", [NF_IN + 1, P], f32)
    a_d = din("a_all", [P, DEPTH], f32)          # col i = gat_a[i][:H]
    WaT_d = din("WaT", [P, DEPTH * P], f32)      # per layer [128,128]
    WbT_d = din("WbT", [P, DEPTH * P], f32)
    nfW1_d = din("nfW1T", [P, P], f32)
    nfW2_d = din("nfW2T", [P, NF_OUT], f32)
    nfb2_d = din("nfb2", [NF_OUT, 1], f32)
    nfg_d = din("nf_g", [P, 1], f32)
    nfbeta_d = din("nf_b", [P, 1], f32)
    gfW1_d = din("gfW1T", [P, P], f32)
    gfW2_d = din("gfW2T", [P, GF_OUT], f32)
    gfb2_d = din("gfb2", [GF_OUT, 1], f32)
    gfg_d = din("gf_g", [P, 1], f32)
    gfbeta_d = din("gf_b", [P, 1], f32)

    nf_outT = nc.dram_tensor("nf_outT", [NF_OUT, NMAX], f32, kind="ExternalOutput").ap()
    gf_outT = nc.dram_tensor("gf_outT", [GF_OUT, GPC], f32, kind="ExternalOutput").ap()

    rg = [list(range(NCORES))]

    with tile.TileContext(nc) as tc:
        with (
            tc.tile_pool(name="dram", bufs=1, space="DRAM") as dp,
            tc.tile_pool(name="cst", bufs=1) as cp,
            tc.tile_pool(name="xin", bufs=2) as xp,
            tc.tile_pool(name="gat", bufs=2) as gp,
            tc.tile_pool(name="mm", bufs=4) as mp,
            tc.tile_pool(name="wrk", bufs=2) as wp,
            tc.tile_pool(name="sml", bufs=4) as sp,
            tc.tile_pool(name="pacc", bufs=2, space="PSUM") as pacc,
            tc.tile_pool(name="ptp", bufs=2, space="PSUM") as ptp,
            tc.tile_pool(name="pbig", bufs=2, space="PSUM") as pbig,
            tc.tile_pool(name="psml", bufs=2, space="PSUM") as psml,
        ):
            table = dp.tile([NTOT, WT], bf16, tag="table")
            g_own = dp.tile([NMAX, WC], bf16, tag="g_own")
            g_all = dp.tile([NTOT, WC], bf16, tag="g_all")
            st_in = dp.tile([P, 4], f32, tag="st_in")
            st_out = dp.tile([P, 4], f32, tag="st_out")

            def load(dst_shape, src_ap, dt, tag):
                t = cp.tile(dst_shape, dt, tag=tag)
                nc.sync.dma_start(t[:], src_ap)
                return t

            idx_s = load([P, S * 8], idx_d[:], i16, "idx")
            dstl_s = load([P, S], dstl_d[:], bf16, "dstl")
            invmsk_s = load([P, NCHUNK], invmsk_d[:], f32, "invmsk")
            P_s = load([P, NTILES * P], P_d[:], bf16, "Pt")
            PT_s = load([P, NTILES * P], PT_d[:], bf16, "PTt")
            iota_s = load([P, P], iota_d[:], bf16, "iota")
            ident_s = load([P, P], ident_d[:], f32, "ident")
            ones_s = load([P, 1], ones_d[:], f32, "ones")
            WiT_s = load([NF_IN + 1, P], WiT_d[:], f32, "WiT")
            a_s = load([P, DEPTH], a_d[:], f32, "a")
            WaT_s = load([P, DEPTH * P], WaT_d[:], f32, "WaT")
            WbT_s = load([P, DEPTH * P], WbT_d[:], f32, "WbT")
            nfW1_s = load([P, P], nfW1_d[:], f32, "nfW1")
            nfW2_s = load([P, NF_OUT], nfW2_d[:], f32, "nfW2")
            nfb2_s = load([NF_OUT, 1], nfb2_d[:], f32, "nfb2")
            nfg_s = load([P, 1], nfg_d[:], f32, "nfg")
            nfbeta_s = load([P, 1], nfbeta_d[:], f32, "nfbeta")
            gfW1_s = load([P, P], gfW1_d[:], f32, "gfW1")
            gfW2_s = load([P, GF_OUT], gfW2_d[:], f32, "gfW2")
            gfb2_s = load([GF_OUT, 1], gfb2_d[:], f32, "gfb2")
            gfg_s = load([P, 1], gfg_d[:], f32, "gfg")
            gfbeta_s = load([P, 1], gfbeta_d[:], f32, "gfbeta")

            nfT_b = cp.tile([P, NMAX], f32, tag="nfT")
            aggT = cp.tile([P, NMAX], f32, tag="aggT")
            nfrow = cp.tile([P, NMAX], bf16, tag="nfrow")
            lbuf = cp.tile([P, NTILES], f32, tag="lbuf")

            def r32(ap):
                return ap

            def g_tile_build(src_sbuf_tile, a_col, g_rows_ap):
                """src_sbuf_tile: [128 feat,128 nodes] f32 slice; writes g rows."""
                ps = psml.tile([P, 1], f32, space="PSUM", tag="s")
                nc.tensor.matmul(ps[:], lhsT=src_sbuf_tile, rhs=a_col,
                                 start=True, stop=True)
                es = sp.tile([P, 1], f32, tag="es")
                nc.scalar.activation(es[:], ps[:], AF.Exp)
                tp = ptp.tile([P, P], f32, space="PSUM", tag="tp")
                nc.tensor.transpose(tp[:], src_sbuf_tile, ident_s[:])
                gt = mp.tile([P, WC], bf16, tag="gt")
                nc.vector.tensor_scalar_mul(gt[:, 0:H], tp[:], es[:, :1])
                nc.vector.tensor_copy(gt[:, H:H + 1], es[:])
                nc.sync.dma_start(g_rows_ap, gt[:])

            # ---- prologue: own nf0 + replicated g0 table ----
            for b in range(NBLK):
                xb = xp.tile([NF_IN + 1, 512], f32, tag="xb")
                nc.sync.dma_start(xb[:], xT_own[:, b * 512:(b + 1) * 512])
                pb = pbig.tile([P, 512], f32, space="PSUM", tag="pb")
                nc.tensor.matmul(pb[:], lhsT=r32(WiT_s[:]), rhs=r32(xb[:]),
                                 start=True, stop=True)
                nc.scalar.activation(nfT_b[:, b * 512:(b + 1) * 512], pb[:], AF.Copy)
            for bb in range(NTOT // 512):
                xb = xp.tile([NF_IN + 1, 512], f32, tag="xb")
                nc.sync.dma_start(xb[:], xT_aug[:, bb * 512:(bb + 1) * 512])
                pb = pbig.tile([P, 512], f32, space="PSUM", tag="pb")
                nc.tensor.matmul(pb[:], lhsT=r32(WiT_s[:]), rhs=r32(xb[:]),
                                 start=True, stop=True)
                blk = wp.tile([P, 512], f32, tag="blk")
                nc.scalar.activation(blk[:], pb[:], AF.Copy)
                for t4 in range(4):
                    r0 = bb * 512 + t4 * P
                    g_tile_build(blk[:, t4 * P:(t4 + 1) * P], a_s[:, 0:1],
                                 table[r0:r0 + P, 0:WC])

            # ---- layers ----
            for i in range(DEPTH):
                cur = new = nfT_b
                # edge phase
                for k in range(NCHUNK):
                    first_sub, nsub_tot, instrs = chunk_meta[k]
                    acc = pacc.tile([P, WC], f32, space="PSUM", tag="acc")
                    sub = first_sub
                    done = 0
                    for (half, nsub) in instrs:
                        gt = gp.tile([P, SUBMAX * WT], bf16, tag="g")
                        g3 = gt[:, :nsub * WT].rearrange("p (n e) -> p n e", n=nsub)
                        in_ap = table[0:SPLIT, :] if half == 0 else table[SPLIT:NTOT, :]
                        nc.gpsimd.dma_gather(
                            out_ap=g3, in_ap=in_ap,
                            idxs_ap=idx_s[:, sub * 8:(sub + nsub) * 8],
                            num_idxs=nsub * P, num_idxs_reg=nsub * P,
                            elem_size=WT, single_packet=False,
                            queue_num=(sub // SUBMAX) % NQ,
                        )
                        for j in range(nsub):
                            M = mp.tile([P, P], bf16, tag="M")
                            nc.vector.tensor_tensor(
                                out=M[:],
                                in0=dstl_s[:, sub + j:sub + j + 1].to_broadcast([P, P]),
                                in1=iota_s[:], op=AL.is_equal)
                            nc.tensor.matmul(
                                acc[:], lhsT=M[:], rhs=g3[:, j, 0:WC],
                                start=(done == 0), stop=(done == nsub_tot - 1))
                            done += 1
                        sub += nsub
                    den = sp.tile([P, 1], f32, tag="den")
                    nc.vector.tensor_tensor(out=den[:], in0=acc[:, H:H + 1],
                                            in1=invmsk_s[:, k:k + 1], op=AL.add)
                    rec = sp.tile([P, 1], f32, tag="rec")
                    nc.vector.reciprocal(rec[:], den[:])
                    arow = wp.tile([P, P], f32, tag="arow")
                    nc.vector.tensor_scalar_mul(arow[:], acc[:, 0:H], rec[:, :1])
                    tp = ptp.tile([P, P], f32, space="PSUM", tag="tp")
                    nc.tensor.transpose(tp[:], arow[:], ident_s[:])
                    nc.scalar.activation(aggT[:, k * P:(k + 1) * P], tp[:], AF.Copy)
                # node phase
                for b in range(NBLK):
                    sl = slice(b * 512, (b + 1) * 512)
                    pb = pbig.tile([P, 512], f32, space="PSUM", tag="pb")
                    nc.tensor.matmul(pb[:], lhsT=r32(WaT_s[:, i * P:(i + 1) * P]),
                                     rhs=r32(cur[:, sl]), start=True, stop=False)
                    nc.tensor.matmul(pb[:], lhsT=r32(WbT_s[:, i * P:(i + 1) * P]),
                                     rhs=r32(aggT[:, sl]), start=False, stop=True)
                    nc.scalar.activation(new[:, sl], pb[:], AF.Relu)
                    if i < DEPTH - 1:
                        for t4 in range(4):
                            t = b * 4 + t4
                            g_tile_build(new[:, t * P:(t + 1) * P],
                                         a_s[:, i + 1:i + 2],
                                         g_own[t * P:(t + 1) * P, :])
                    else:
                        sq = wp.tile([P, 512], f32, tag="sq")
                        nc.scalar.activation(sq[:], new[:, sl], AF.Square)
                        for t4 in range(4):
                            t = b * 4 + t4
                            tp = ptp.tile([P, P], f32, space="PSUM", tag="tp")
                            nc.tensor.transpose(tp[:], new[:, t * P:(t + 1) * P],
                                                ident_s[:])
                            nc.vector.tensor_copy(nfrow[:, t * P:(t + 1) * P], tp[:])
                            ps = psml.tile([P, 1], f32, space="PSUM", tag="s")
                            nc.tensor.matmul(ps[:], lhsT=sq[:, t4 * P:(t4 + 1) * P],
                                             rhs=ones_s[:], start=True, stop=True)
                            nc.vector.tensor_copy(lbuf[:, t:t + 1], ps[:])
                if i < DEPTH - 1:
                    nc.gpsimd.collective_compute(
                        "AllGather", AL.bypass, ins=[g_own.opt()],
                        outs=[g_all.opt()], replica_groups=rg)
                    for j in range(NCORES):
                        rs = slice(j * NMAX, (j + 1) * NMAX)
                        nc.sync.dma_start(table[rs, 0:WC], g_all[rs, :])

            # ---- pooling (3 iterations, all local) ----
            esf = cp.tile([P, NTILES], f32, tag="esf")
            bbuf = cp.tile([P, NTILES], f32, tag="bbuf")
            alpha = cp.tile([P, NTILES], f32, tag="alpha")
            curl = lbuf
            for it in range(3):
                nc.scalar.activation(esf[:], curl[:], AF.Exp)
                esb = wp.tile([P, NTILES], bf16, tag="esb")
                nc.vector.tensor_copy(esb[:], esf[:])
                Sp = psml.tile([P, 1], f32, space="PSUM", tag="s")
                for t in range(NTILES):
                    nc.tensor.matmul(Sp[:], lhsT=P_s[:, t * P:(t + 1) * P],
                                     rhs=esb[:, t:t + 1], start=(t == 0),
                                     stop=(t == NTILES - 1))
                r = sp.tile([P, 1], f32, tag="r")
                nc.vector.reciprocal(r[:], Sp[:])
                rb = sp.tile([P, 1], bf16, tag="rb")
                nc.vector.tensor_copy(rb[:], r[:])
                for t in range(NTILES):
                    pb2 = psml.tile([P, 1], f32, space="PSUM", tag="s")
                    nc.tensor.matmul(pb2[:], lhsT=PT_s[:, t * P:(t + 1) * P],
                                     rhs=rb[:], start=True, stop=True)
                    nc.vector.tensor_copy(bbuf[:, t:t + 1], pb2[:])
                nc.vector.tensor_tensor(out=alpha[:], in0=esf[:], in1=bbuf[:],
                                        op=AL.mult)
                if it < 2:
                    nl = cp.tile([P, NTILES], f32, tag=f"l{it}")
                    nc.vector.tensor_tensor(out=nl[:], in0=alpha[:], in1=lbuf[:],
                                            op=AL.mult)
                    curl = nl
            gfacc = ptp.tile([P, P], f32, space="PSUM", tag="tp")
            for t in range(NTILES):
                Mp = mp.tile([P, P], bf16, tag="M")
                nc.vector.tensor_scalar_mul(Mp[:], P_s[:, t * P:(t + 1) * P],
                                            alpha[:, t:t + 1])
                nc.tensor.matmul(gfacc[:], lhsT=Mp[:],
                                 rhs=nfrow[:, t * P:(t + 1) * P],
                                 start=(t == 0), stop=(t == NTILES - 1))
            gf_row = wp.tile([P, P], f32, tag="gf_row")
            nc.scalar.activation(gf_row[:], gfacc[:], AF.Copy)

            # ---- heads: stats pass ----
            nfT3 = nfT_b
            s1c = cp.tile([P, NBLK], f32, tag="s1c")
            s2c = cp.tile([P, NBLK], f32, tag="s2c")
            for b in range(NBLK):
                sl = slice(b * 512, (b + 1) * 512)
                pb = pbig.tile([P, 512], f32, space="PSUM", tag="pb")
                nc.tensor.matmul(pb[:], lhsT=r32(nfW1_s[:]), rhs=r32(nfT3[:, sl]),
                                 start=True, stop=True)
                jnk = wp.tile([P, 512], f32, tag="jnk")
                nc.scalar.activation(jnk[:], pb[:], AF.Copy,
                                     accum_out=s1c[:, b:b + 1])
                jnk2 = wp.tile([P, 512], f32, tag="jnk")
                nc.scalar.activation(jnk2[:], pb[:], AF.Square,
                                     accum_out=s2c[:, b:b + 1])
            nfS1 = sp.tile([P, 1], f32, tag="nfS1")
            nfS2 = sp.tile([P, 1], f32, tag="nfS2")
            nc.vector.reduce_sum(nfS1[:], s1c[:], axis=mybir.AxisListType.X)
            nc.vector.reduce_sum(nfS2[:], s2c[:], axis=mybir.AxisListType.X)

            tpg = ptp.tile([P, P], f32, space="PSUM", tag="tp")
            nc.tensor.transpose(tpg[:], gf_row[:], ident_s[:])
            gfT_s = wp.tile([P, P], f32, tag="gfT")
            nc.scalar.activation(gfT_s[:], tpg[:], AF.Copy)
            pz = ptp.tile([P, P], f32, space="PSUM", tag="tp")
            nc.tensor.matmul(pz[:], lhsT=gfT_s[:], rhs=gfW1_s[:],
                             start=True, stop=True)
            z_row = wp.tile([P, P], f32, tag="z_row")
            nc.scalar.activation(z_row[:], pz[:], AF.Copy)
            pzT = ptp.tile([P, P], f32, space="PSUM", tag="tp")
            nc.tensor.transpose(pzT[:], z_row[:], ident_s[:])
            zT_s = cp.tile([P, P], f32, tag="zT")
            gfS1 = sp.tile([P, 1], f32, tag="gfS1")
            nc.scalar.activation(zT_s[:], pzT[:], AF.Copy, accum_out=gfS1[:])
            zjnk = wp.tile([P, P], f32, tag="jnk")
            gfS2 = sp.tile([P, 1], f32, tag="gfS2")
            nc.scalar.activation(zjnk[:], pzT[:], AF.Square, accum_out=gfS2[:])

            stat = wp.tile([P, 4], f32, tag="stat")
            for col, t_ in enumerate((nfS1, nfS2, gfS1, gfS2)):
                nc.vector.tensor_copy(stat[:, col:col + 1], t_[:])
            nc.sync.dma_start(st_in[:], stat[:])
            nc.gpsimd.collective_compute("AllReduce", AL.add, ins=[st_in.opt()],
                                       outs=[st_out.opt()], replica_groups=rg)
            statr = wp.tile([P, 4], f32, tag="statr")
            nc.sync.dma_start(statr[:], st_out[:])

            def bn_coeffs(S1col, S2col, inv_n, gamma, beta, tag):
                mu = sp.tile([P, 1], f32, tag=tag + "mu")
                nc.vector.tensor_scalar_mul(mu[:], S1col, inv_n)
                ex2 = sp.tile([P, 1], f32, tag=tag + "e2")
                nc.vector.tensor_scalar_mul(ex2[:], S2col, inv_n)
                musq = sp.tile([P, 1], f32, tag=tag + "ms")
                nc.vector.tensor_tensor(out=musq[:], in0=mu[:], in1=mu[:], op=AL.mult)
                var = sp.tile([P, 1], f32, tag=tag + "var")
                nc.vector.tensor_tensor(out=var[:], in0=ex2[:], in1=musq[:],
                                        op=AL.subtract)
                vre = sp.tile([P, 1], f32, tag=tag + "vre")
                nc.vector.tensor_scalar_add(vre[:], var[:], EPS)
                rv = sp.tile([P, 1], f32, tag=tag + "rv")
                nc.vector.reciprocal(rv[:], vre[:])
                rs = sp.tile([P, 1], f32, tag=tag + "rs")
                nc.scalar.activation(rs[:], rv[:], AF.Sqrt)
                sc = sp.tile([P, 1], f32, tag=tag + "sc")
                nc.vector.tensor_tensor(out=sc[:], in0=rs[:], in1=gamma, op=AL.mult)
                msc = sp.tile([P, 1], f32, tag=tag + "msc")
                nc.vector.tensor_tensor(out=msc[:], in0=mu[:], in1=sc[:], op=AL.mult)
                bi = sp.tile([P, 1], f32, tag=tag + "bi")
                nc.vector.tensor_tensor(out=bi[:], in0=beta, in1=msc[:],
                                        op=AL.subtract)
                return sc, bi

            nsc, nbi = bn_coeffs(statr[:, 0:1], statr[:, 1:2], 1.0 / N,
                                 nfg_s[:], nfbeta_s[:], "n")
            gsc, gbi = bn_coeffs(statr[:, 2:3], statr[:, 3:4], 1.0 / G,
                                 gfg_s[:], gfbeta_s[:], "g")

            # nf head final pass
            for b in range(NBLK):
                sl = slice(b * 512, (b + 1) * 512)
                pb = pbig.tile([P, 512], f32, space="PSUM", tag="pb")
                nc.tensor.matmul(pb[:], lhsT=r32(nfW1_s[:]), rhs=r32(nfT3[:, sl]),
                                 start=True, stop=True)
                ybn = wp.tile([P, 512], f32, tag="ybn")
                nc.scalar.activation(ybn[:], pb[:], AF.Relu, bias=nbi[:], scale=nsc[:])
                po = pbig.tile([NF_OUT, 512], f32, space="PSUM", tag="pb")
                nc.tensor.matmul(po[:], lhsT=r32(nfW2_s[:]), rhs=r32(ybn[:]),
                                 start=True, stop=True)
                ob = wp.tile([NF_OUT, 512], f32, tag="ob")
                nc.vector.tensor_scalar_add(ob[:], po[:], nfb2_s[:, :1])
                nc.sync.dma_start(nf_outT[:, sl], ob[:])

            # gf head final
            zbn = wp.tile([P, P], f32, tag="zbn")
            nc.scalar.activation(zbn[:], zT_s[:], AF.Relu, bias=gbi[:], scale=gsc[:])
            pgo = ptp.tile([GF_OUT, GPC], f32, space="PSUM", tag="tp")
            nc.tensor.matmul(pgo[:], lhsT=gfW2_s[:], rhs=zbn[:, 0:GPC],
                             start=True, stop=True)
            og = wp.tile([GF_OUT, GPC], f32, tag="og")
            nc.vector.tensor_scalar_add(og[:], pgo[:], gfb2_s[:, :1])
            nc.sync.dma_start(gf_outT[:], og[:])

    nc.compile()
    return nc


def kernel(**inputs):
    global LAST_EXEC_NS
    _install_hw_shims()

    x = np.asarray(inputs["x"], np.float32)
    edge_index = np.asarray(inputs["edge_index"])
    edge_attr = np.asarray(inputs["edge_attr"])
    batch = np.asarray(inputs["batch"], np.int64)

    sched = _schedule(edge_index, batch)
    gb = sched["gb"]

    nc = _build(sched)

    # host-side constant prep
    xT_aug = np.zeros((NF_IN + 1, NTOT), np.float32)
    for c in range(NCORES):
        n_ = int(sched["n_c"][c])
        seg = x[gb[c]:gb[c + 1]]
        xT_aug[:NF_IN, c * NMAX:c * NMAX + n_] = seg.T
        xT_aug[NF_IN, c * NMAX:c * NMAX + n_] = 1.0
    iota = np.tile(np.arange(P, dtype=np.float32)[None, :], (P, 1)).astype(
        ml_dtypes.bfloat16)
    ident = np.eye(P, dtype=np.float32)
    ones = np.ones((P, 1), np.float32)
    gat_a = np.asarray(inputs["gat_a"], np.float32)
    gat_W = np.asarray(inputs["gat_W"], np.float32)
    a_all = gat_a[:, :H].T.copy()                      # [128, 3]
    WaT = np.concatenate([gat_W[i].T[:H] for i in range(DEPTH)], 1)
    WbT = np.concatenate([gat_W[i].T[H:] for i in range(DEPTH)], 1)
    WiT = np.concatenate([np.asarray(inputs["Wi"], np.float32).T,
                          np.asarray(inputs["bi"], np.float32)[None, :]], 0)

    common = dict(
        xT_aug=xT_aug, iota=iota, ident=ident, ones=ones,
        WiT=WiT, a_all=a_all, WaT=WaT, WbT=WbT,
        nfW1T=np.asarray(inputs["nfW1"], np.float32).T.copy(),
        nfW2T=np.asarray(inputs["nfW2"], np.float32).T.copy(),
        nfb2=np.asarray(inputs["nfb2"], np.float32)[:, None],
        nf_g=np.asarray(inputs["nf_g"], np.float32)[:, None],
        nf_b=np.asarray(inputs["nf_b"], np.float32)[:, None],
        gfW1T=np.asarray(inputs["gfW1"], np.float32).T.copy(),
        gfW2T=np.asarray(inputs["gfW2"], np.float32).T.copy(),
        gfb2=np.asarray(inputs["gfb2"], np.float32)[:, None],
        gf_g=np.asarray(inputs["gf_g"], np.float32)[:, None],
        gf_b=np.asarray(inputs["gf_b"], np.float32)[:, None],
    )
    in_maps = []
    for c in range(NCORES):
        m = dict(common)
        m["xT_own"] = np.ascontiguousarray(xT_aug[:, c * NMAX:(c + 1) * NMAX])
        m["idx16"] = sched["idx_streams"][c]
        m["dstl"] = sched["dstl_streams"][c]
        m["invmsk"] = sched["invmsk"][c]
        m["Pt"] = sched["Ptiles"][c]
        m["PTt"] = sched["PTtiles"][c]
        in_maps.append(m)

    trace = os.environ.get("KERNEL_TRACE", "0") == "1"
    try:
        res = bass_utils.run_bass_kernel_spmd(
            nc, in_maps, core_ids=list(range(NCORES)), trace=trace)
    except Exception:
        if not trace:
            raise
        res = bass_utils.run_bass_kernel_spmd(
            nc, in_maps, core_ids=list(range(NCORES)), trace=False)
    LAST_EXEC_NS = res.exec_time_ns

    nf_out = np.zeros((N, NF_OUT), np.float32)
    for c in range(NCORES):
        n_ = int(sched["n_c"][c])
        nf_out[gb[c]:gb[c + 1]] = res.results[c]["nf_outT"][:, :n_].T
    gf_out = np.zeros((G, GF_OUT), np.float32)
    for c in range(NCORES):
        gf_out[c * GPC:(c + 1) * GPC] = res.results[c]["gf_outT"].T
    return nf_out, edge_attr, gf_out
